# revision 22
# baseline (speedup 1.0000x reference)
"""Bahdanau-attention score kernel (softmax(v . tanh(W[h;enc]+b))) for 8 TRN2 cores.

Self-contained: hardcodes shapes B=32, S=2048, ENC2=600, DD=900.
Sharding: data-parallel over batch (4 batches/core), weights replicated.

Design (v6):
- Host prep: enc is cast to fp16 and TRANSPOSED on the host into
  encT [604, 8192] (rows 600..603 carry a one-hot(batch), so the
  h-projection -- computed on host and appended to We^T rows -- lands
  via 4 extra contraction rows). Host transposition lets every device
  load be a plain row-major DMA: measured ~178 GB/s effective vs
  ~76 GB/s for the on-chip xbar-transpose path (which also corrupts
  its 512-row tail and serializes per queue).
- PE runs the 10 contraction matmuls per 128-row tile (fp16 operands,
  f32 PSUM, N split 512+388 for the PSUM bank limit).
- ACT drains PSUM with tanh (fp16 out); DVE does one fused
  scalar_tensor_tensor: junk=(z*1)*v_rep with accum_out=scores column.
- Softmax is per batch and overlapped: transpose+exp after the batch's
  last tile, normalize via PE ones-matmul sum + reciprocal + PE
  broadcast (no DRAM roundtrip), per-batch output DMA on the idle
  GPSIMD queue. Only batch 3's tail is exposed.
"""

import os

import numpy as np

import concourse.bass as bass  # noqa: F401
import concourse.mybir as mybir
import concourse.tile as tile
from concourse import bacc
from concourse.bass_utils import run_bass_kernel_spmd
from concourse.masks import make_identity

F32 = mybir.dt.float32
F16 = mybir.dt.float16
AF = mybir.ActivationFunctionType
ALU = mybir.AluOpType
AX = mybir.AxisListType

NCORES = 8
B, S, E2, DD = 32, 2048, 600, 900
BL = B // NCORES            # 4 batches per core
SROWS = BL * S              # 8192 s-rows per core
P = 128
NTIL = SROWS // P           # 64 score tiles/columns
TPB = S // P                # 16 tiles per batch
NCH = 5                     # e chunks of 128 (last: 88 enc + 4 one-hot)
K4 = 92                     # chunk-4 contraction rows
KE = 512 + K4               # 604 encT rows

K_TILES = int(os.environ.get("K_TILES", NTIL))


def build():
    nc = bacc.Bacc("TRN2", target_bir_lowering=False)
    encT_ext = nc.dram_tensor("encT", [KE, SROWS], F16, kind="ExternalInput")
    wcat_ext = nc.dram_tensor("wcat", [KE, DD], F16, kind="ExternalInput")
    v_ext = nc.dram_tensor("v", [1, DD], F16, kind="ExternalInput")
    out_ext = nc.dram_tensor("out", [BL, S], F32, kind="ExternalOutput")

    with tile.TileContext(nc) as tc:
        with (
            tc.tile_pool(name="stat", bufs=1) as stat,
            tc.tile_pool(name="encp", bufs=BL) as encp,
            tc.tile_pool(name="zp", bufs=3) as zp,
            tc.tile_pool(name="jp", bufs=2) as jp,
            tc.tile_pool(name="ps_e", bufs=3, space="PSUM") as ps_e,
            tc.tile_pool(name="ps_t", bufs=2, space="PSUM") as ps_t,
        ):
            # --------- weights split across the three DMA queue heads -----
            rhs_main = stat.tile([P, 4, DD], F16)
            rhs4 = stat.tile([K4, DD], F16)
            v_sb = stat.tile([1, DD], F16)
            v_rep = stat.tile([P, DD], F16)
            ones_r = stat.tile([1, P], F16)
            nc.vector.memset(ones_r[:, :], 1.0)

            def w_dma(eng, h):
                # halves so the single-transfer latency (~22 GB/s/engine)
                # doesn't gate the first tiles
                for (no, nn) in ((0, 450), (450, 450)):
                    eng.dma_start(out=rhs_main[:, h, no:no + nn],
                                  in_=wcat_ext.ap()[h * P:(h + 1) * P,
                                                    no:no + nn])

            nc.gpsimd.dma_start(out=v_sb[:, :], in_=v_ext.ap())
            w_dma(nc.sync, 0)
            for (no, nn) in ((0, 450), (450, 450)):
                nc.sync.dma_start(out=rhs4[:, no:no + nn],
                                  in_=wcat_ext.ap()[512:KE, no:no + nn])
            w_dma(nc.scalar, 1)
            w_dma(nc.scalar, 2)
            w_dma(nc.gpsimd, 3)

            # v broadcast across partitions: psum[p, :] = ones[p] * v
            vps = ps_e.tile([P, DD], F32, tag="ep")
            for (no, nn) in ((0, 512), (512, 388)):
                nc.tensor.matmul(vps[:, no:no + nn], ones_r[:, :],
                                 v_sb[:, no:no + nn])
            nc.scalar.copy(v_rep[:, :], vps[:, :])

            # --------- enc tiles: plain row-major loads on SP -------------
            # Early batches are split into column pieces: a whole [128,2048]
            # transfer has ~20us single-engine latency, which would gate the
            # first tiles even though aggregate bandwidth is fine.
            enc_tiles = {}
            col_splits = {
                0: (256, 1024, S),
                1: (1024, S),
            }
            for b in range(BL):
                for c in range(NCH):
                    r0, rn = (c * P, P) if c < 4 else (512, K4)
                    et = encp.tile([P, S], F16, tag=f"enc{c}",
                                   name=f"enc{b}_{c}")
                    s0 = 0
                    for s1 in col_splits.get(b, (S,)):
                        nc.sync.dma_start(
                            out=et[0:rn, s0:s1],
                            in_=encT_ext.ap()[r0:r0 + rn,
                                              b * S + s0:b * S + s1],
                        )
                        s0 = s1
                    enc_tiles[(b, c)] = et

            # ---------------- constants ----------------
            ident_f = stat.tile([P, P], F32)
            make_identity(nc, ident_f[:, :])
            ones16 = stat.tile([TPB, 1], F32)
            nc.vector.memset(ones16[:, :], 1.0)
            ones1x16 = stat.tile([1, TPB], F32)
            nc.vector.memset(ones1x16[:, :], 1.0)

            scores = stat.tile([P, NTIL], F32)
            e1 = stat.tile([TPB, BL, P], F32)
            rs = stat.tile([TPB, BL], F32)
            rbi = stat.tile([1, BL], F32)
            outf = stat.tile([TPB, BL, P], F32)

            # ---------------- per-batch softmax pieces ----------------
            def emit_exp(b):
                c0 = b * TPB
                pst = ps_t.tile([P, P], F32, tag="tp", name=f"pst{b}")
                nc.tensor.transpose(
                    pst[0:TPB, :], scores[:, c0:c0 + TPB], ident_f[:, :]
                )
                nc.scalar.activation(
                    e1[:, b, :], pst[0:TPB, :], AF.Exp,
                    accum_out=rs[:, b:b + 1],
                )

            def emit_tail(b):
                zb = ps_t.tile([P, P], F32, tag="tp", name=f"zb{b}")
                nc.tensor.matmul(zb[0:1, 0:1], ones16[:, :], rs[:, b:b + 1])
                nc.vector.reciprocal(rbi[:, b:b + 1], zb[0:1, 0:1])
                rfacp = ps_t.tile([P, P], F32, tag="tp", name=f"rf{b}")
                nc.tensor.matmul(rfacp[0:TPB, 0:1], ones1x16[:, :],
                                 rbi[:, b:b + 1])
                nc.vector.tensor_scalar_mul(
                    outf[:, b, :], e1[:, b, :], rfacp[0:TPB, 0:1]
                )
                nc.gpsimd.dma_start(
                    out=out_ext.ap()[b:b + 1, :].rearrange(
                        "b (t p) -> (b t) p", p=P),
                    in_=outf[:, b, :],
                )

            # ---------------- main loop ----------------
            for t in range(K_TILES):
                b, ti = divmod(t, TPB)
                eps = ps_e.tile([P, DD], F32, tag="ep")
                for c in range(NCH):
                    et = enc_tiles[(b, c)]
                    kk = P if c < 4 else K4
                    rr = rhs_main[:, c, :] if c < 4 else rhs4[:, :]
                    for (no, nn) in ((0, 512), (512, 388)):
                        nc.tensor.matmul(
                            eps[:, no:no + nn],
                            et[0:kk, ti * P:(ti + 1) * P],
                            rr[:, no:no + nn],
                            start=(c == 0), stop=(c == NCH - 1),
                        )
                z = zp.tile([P, DD], F16, tag="z")
                nc.scalar.activation(z[:, :], eps[:, :], AF.Tanh)
                junk = jp.tile([P, DD], F16, tag="junk")
                nc.vector.scalar_tensor_tensor(
                    out=junk[:, :], in0=z[:, :], scalar=1.0, in1=v_rep[:, :],
                    op0=ALU.mult, op1=ALU.mult,
                    accum_out=scores[:, t:t + 1],
                )

                if K_TILES != NTIL:
                    continue
                # overlapped softmax for the previous batch
                if b >= 1 and ti == 1:
                    emit_exp(b - 1)
                if b >= 1 and ti == 6:
                    emit_tail(b - 1)

            if K_TILES < NTIL:
                return nc

            emit_exp(BL - 1)
            emit_tail(BL - 1)
    return nc


_CACHE = {}


def _get_nc():
    if "nc" not in _CACHE:
        nc = build()
        nc.compile()
        _CACHE["nc"] = nc
    return _CACHE["nc"]


def make_in_maps(hidden, encoder_outputs, attn_W, attn_b, v):
    hidden = np.asarray(hidden, dtype=np.float32)
    encoder_outputs = np.asarray(encoder_outputs, dtype=np.float32)
    attn_W = np.asarray(attn_W, dtype=np.float32)
    attn_b = np.asarray(attn_b, dtype=np.float32)
    v = np.asarray(v, dtype=np.float32)

    WeT = np.ascontiguousarray(attn_W[:, DD:].T)          # [600, 900]
    hb_all = hidden @ attn_W[:, :DD].T + attn_b           # [32, 900]
    v16 = v.astype(np.float16).reshape(1, DD)

    in_maps = []
    for c in range(NCORES):
        bs = slice(c * BL, (c + 1) * BL)
        encT = np.zeros((KE, SROWS), dtype=np.float16)
        encT[:E2, :] = encoder_outputs[bs].reshape(SROWS, E2).T
        for b in range(BL):
            encT[E2 + b, b * S:(b + 1) * S] = 1.0
        wcat = np.concatenate([WeT, hb_all[bs]], axis=0).astype(np.float16)
        in_maps.append({
            "encT": encT,
            "wcat": np.ascontiguousarray(wcat),
            "v": v16,
        })
    return in_maps


def run(in_maps, trace=False, **kw):
    nc = _get_nc()
    return run_bass_kernel_spmd(nc, in_maps, core_ids=list(range(NCORES)),
                                trace=trace, **kw)


def kernel(hidden, encoder_outputs, attn_W, attn_b, v):
    in_maps = make_in_maps(hidden, encoder_outputs, attn_W, attn_b, v)
    try:
        res = run(in_maps)
    except Exception:
        # transient device states (e.g. a previously wedged core) sometimes
        # clear on retry
        res = run(in_maps)
    out = np.concatenate([res.results[c]["out"] for c in range(NCORES)], axis=0)
    return np.ascontiguousarray(out, dtype=np.float32)


# revision 23
# speedup vs baseline: 1.1699x; 1.1699x over previous
"""Bahdanau-attention score kernel (softmax(v . tanh(W[h;enc]+b))) for 8 TRN2 cores.

Self-contained: hardcodes shapes B=32, S=2048, ENC2=600, DD=900.
Sharding: data-parallel over batch (4 batches/core), weights replicated.

Design (v6):
- Host prep: enc is cast to fp16 and TRANSPOSED on the host into
  encT [604, 8192] (rows 600..603 carry a one-hot(batch), so the
  h-projection -- computed on host and appended to We^T rows -- lands
  via 4 extra contraction rows). Host transposition lets every device
  load be a plain row-major DMA: measured ~178 GB/s effective vs
  ~76 GB/s for the on-chip xbar-transpose path (which also corrupts
  its 512-row tail and serializes per queue).
- PE runs the 10 contraction matmuls per 128-row tile (fp16 operands,
  f32 PSUM, N split 512+388 for the PSUM bank limit).
- ACT drains PSUM with tanh (fp16 out); DVE does one fused
  scalar_tensor_tensor: junk=(z*1)*v_rep with accum_out=scores column.
- Softmax is per batch and overlapped: transpose+exp after the batch's
  last tile, normalize via PE ones-matmul sum + reciprocal + PE
  broadcast (no DRAM roundtrip), per-batch output DMA on the idle
  GPSIMD queue. Only batch 3's tail is exposed.
"""

import os

import numpy as np

import concourse.bass as bass  # noqa: F401
import concourse.mybir as mybir
import concourse.tile as tile
from concourse import bacc
from concourse.bass_utils import run_bass_kernel_spmd
from concourse.masks import make_identity

F32 = mybir.dt.float32
F16 = mybir.dt.float16
AF = mybir.ActivationFunctionType
ALU = mybir.AluOpType
AX = mybir.AxisListType

NCORES = 8
B, S, E2, DD = 32, 2048, 600, 900
BL = B // NCORES            # 4 batches per core
SROWS = BL * S              # 8192 s-rows per core
P = 128
NTIL = SROWS // P           # 64 score tiles/columns
TPB = S // P                # 16 tiles per batch
NCH = 5                     # e chunks of 128 (last: 88 enc + 4 one-hot)
K4 = 92                     # chunk-4 contraction rows
KE = 512 + K4               # 604 encT rows

K_TILES = int(os.environ.get("K_TILES", NTIL))


def build():
    nc = bacc.Bacc("TRN2", target_bir_lowering=False)
    encT_ext = nc.dram_tensor("encT", [KE, SROWS], F16, kind="ExternalInput")
    wcat_ext = nc.dram_tensor("wcat", [KE, DD], F16, kind="ExternalInput")
    v_ext = nc.dram_tensor("v", [1, DD], F16, kind="ExternalInput")
    out_ext = nc.dram_tensor("out", [BL, S], F32, kind="ExternalOutput")

    with tile.TileContext(nc) as tc:
        with (
            tc.tile_pool(name="stat", bufs=1) as stat,
            tc.tile_pool(name="encp", bufs=BL) as encp,
            tc.tile_pool(name="zp", bufs=3) as zp,
            tc.tile_pool(name="jp", bufs=2) as jp,
            tc.tile_pool(name="ps_e", bufs=3, space="PSUM") as ps_e,
            tc.tile_pool(name="ps_t", bufs=2, space="PSUM") as ps_t,
        ):
            # --------- weights split across the three DMA queue heads -----
            rhs_main = stat.tile([P, 4, DD], F16)
            rhs4 = stat.tile([K4, DD], F16)
            v_sb = stat.tile([1, DD], F16)
            v_rep = stat.tile([P, DD], F16)
            ones_r = stat.tile([1, P], F16)
            nc.vector.memset(ones_r[:, :], 1.0)

            def w_dma(eng, h):
                # halves so the single-transfer latency (~22 GB/s/engine)
                # doesn't gate the first tiles
                for (no, nn) in ((0, 450), (450, 450)):
                    eng.dma_start(out=rhs_main[:, h, no:no + nn],
                                  in_=wcat_ext.ap()[h * P:(h + 1) * P,
                                                    no:no + nn])

            nc.gpsimd.dma_start(out=v_sb[:, :], in_=v_ext.ap())
            w_dma(nc.sync, 0)
            for (no, nn) in ((0, 450), (450, 450)):
                nc.sync.dma_start(out=rhs4[:, no:no + nn],
                                  in_=wcat_ext.ap()[512:KE, no:no + nn])
            nc.sync.dma_start(out=rhs_main[:, 3, 0:450],
                              in_=wcat_ext.ap()[3 * P:4 * P, 0:450])
            w_dma(nc.scalar, 1)
            w_dma(nc.scalar, 2)
            nc.scalar.dma_start(out=rhs_main[:, 3, 450:DD],
                                in_=wcat_ext.ap()[3 * P:4 * P, 450:DD])

            # PE warm-up: keep the HAM activity monitor busy during the
            # startup DMA wait so the first real matmuls run at 2.4 GHz
            warm = ps_t.tile([P, P], F32, tag="tp", name="warm")
            for _ in range(36):
                nc.tensor.matmul(warm[0:P, 0:P], ones_r[:, :], ones_r[:, :])

            # v broadcast across partitions: psum[p, :] = ones[p] * v
            vps = ps_e.tile([P, DD], F32, tag="ep")
            for (no, nn) in ((0, 512), (512, 388)):
                nc.tensor.matmul(vps[:, no:no + nn], ones_r[:, :],
                                 v_sb[:, no:no + nn])
            nc.scalar.copy(v_rep[:, :], vps[:, :])

            # --------- enc tiles: plain row-major loads on SP -------------
            # Early batches are split into column pieces: a whole [128,2048]
            # transfer has ~20us single-engine latency, which would gate the
            # first tiles even though aggregate bandwidth is fine.
            enc_tiles = {}
            col_splits = {
                0: (256, 1024, S),
                1: (1024, S),
            }
            for b in range(BL):
                for c in range(NCH):
                    r0, rn = (c * P, P) if c < 4 else (512, K4)
                    et = encp.tile([P, S], F16, tag=f"enc{c}",
                                   name=f"enc{b}_{c}")
                    s0 = 0
                    for s1 in col_splits.get(b, (S,)):
                        nc.sync.dma_start(
                            out=et[0:rn, s0:s1],
                            in_=encT_ext.ap()[r0:r0 + rn,
                                              b * S + s0:b * S + s1],
                        )
                        s0 = s1
                    enc_tiles[(b, c)] = et

            # ---------------- constants ----------------
            ident_f = stat.tile([P, P], F32)
            make_identity(nc, ident_f[:, :])
            ones16 = stat.tile([TPB, 1], F32)
            nc.vector.memset(ones16[:, :], 1.0)
            ones1x16 = stat.tile([1, TPB], F32)
            nc.vector.memset(ones1x16[:, :], 1.0)

            scores = stat.tile([P, NTIL], F32)
            e1 = stat.tile([TPB, BL, P], F32)
            rs = stat.tile([TPB, BL], F32)
            rbi = stat.tile([1, BL], F32)
            outf = stat.tile([TPB, BL, P], F32)

            # ---------------- per-batch softmax pieces ----------------
            def emit_exp(b):
                c0 = b * TPB
                pst = ps_t.tile([P, P], F32, tag="tp", name=f"pst{b}")
                nc.tensor.transpose(
                    pst[0:TPB, :], scores[:, c0:c0 + TPB], ident_f[:, :]
                )
                nc.scalar.activation(
                    e1[:, b, :], pst[0:TPB, :], AF.Exp,
                    accum_out=rs[:, b:b + 1],
                )

            def emit_tail(b):
                zb = ps_t.tile([P, P], F32, tag="tp", name=f"zb{b}")
                nc.tensor.matmul(zb[0:1, 0:1], ones16[:, :], rs[:, b:b + 1])
                nc.vector.reciprocal(rbi[:, b:b + 1], zb[0:1, 0:1])
                rfacp = ps_t.tile([P, P], F32, tag="tp", name=f"rf{b}")
                nc.tensor.matmul(rfacp[0:TPB, 0:1], ones1x16[:, :],
                                 rbi[:, b:b + 1])
                nc.vector.tensor_scalar_mul(
                    outf[:, b, :], e1[:, b, :], rfacp[0:TPB, 0:1]
                )
                nc.gpsimd.dma_start(
                    out=out_ext.ap()[b:b + 1, :].rearrange(
                        "b (t p) -> (b t) p", p=P),
                    in_=outf[:, b, :],
                )

            # ---------------- main loop ----------------
            for t in range(K_TILES):
                b, ti = divmod(t, TPB)
                eps = ps_e.tile([P, DD], F32, tag="ep")
                for c in range(NCH):
                    et = enc_tiles[(b, c)]
                    kk = P if c < 4 else K4
                    rr = rhs_main[:, c, :] if c < 4 else rhs4[:, :]
                    for (no, nn) in ((0, 512), (512, 388)):
                        nc.tensor.matmul(
                            eps[:, no:no + nn],
                            et[0:kk, ti * P:(ti + 1) * P],
                            rr[:, no:no + nn],
                            start=(c == 0), stop=(c == NCH - 1),
                        )
                z = zp.tile([P, DD], F16, tag="z")
                nc.scalar.activation(z[:, :], eps[:, :], AF.Tanh)
                junk = jp.tile([P, DD], F16, tag="junk")
                nc.vector.scalar_tensor_tensor(
                    out=junk[:, :], in0=z[:, :], scalar=1.0, in1=v_rep[:, :],
                    op0=ALU.mult, op1=ALU.mult,
                    accum_out=scores[:, t:t + 1],
                )

                if K_TILES != NTIL:
                    continue
                # overlapped softmax for the previous batch
                if b >= 1 and ti == 1:
                    emit_exp(b - 1)
                if b >= 1 and ti == 6:
                    emit_tail(b - 1)

            if K_TILES < NTIL:
                return nc

            emit_exp(BL - 1)
            emit_tail(BL - 1)
    return nc


_CACHE = {}


def _get_nc():
    if "nc" not in _CACHE:
        nc = build()
        nc.compile()
        _CACHE["nc"] = nc
    return _CACHE["nc"]


def make_in_maps(hidden, encoder_outputs, attn_W, attn_b, v):
    hidden = np.asarray(hidden, dtype=np.float32)
    encoder_outputs = np.asarray(encoder_outputs, dtype=np.float32)
    attn_W = np.asarray(attn_W, dtype=np.float32)
    attn_b = np.asarray(attn_b, dtype=np.float32)
    v = np.asarray(v, dtype=np.float32)

    WeT = np.ascontiguousarray(attn_W[:, DD:].T)          # [600, 900]
    hb_all = hidden @ attn_W[:, :DD].T + attn_b           # [32, 900]
    v16 = v.astype(np.float16).reshape(1, DD)

    in_maps = []
    for c in range(NCORES):
        bs = slice(c * BL, (c + 1) * BL)
        encT = np.zeros((KE, SROWS), dtype=np.float16)
        encT[:E2, :] = encoder_outputs[bs].reshape(SROWS, E2).T
        for b in range(BL):
            encT[E2 + b, b * S:(b + 1) * S] = 1.0
        wcat = np.concatenate([WeT, hb_all[bs]], axis=0).astype(np.float16)
        in_maps.append({
            "encT": encT,
            "wcat": np.ascontiguousarray(wcat),
            "v": v16,
        })
    return in_maps


def run(in_maps, trace=False, **kw):
    nc = _get_nc()
    return run_bass_kernel_spmd(nc, in_maps, core_ids=list(range(NCORES)),
                                trace=trace, **kw)


def kernel(hidden, encoder_outputs, attn_W, attn_b, v):
    in_maps = make_in_maps(hidden, encoder_outputs, attn_W, attn_b, v)
    try:
        res = run(in_maps)
    except Exception:
        # transient device states (e.g. a previously wedged core) sometimes
        # clear on retry
        res = run(in_maps)
    out = np.concatenate([res.results[c]["out"] for c in range(NCORES)], axis=0)
    return np.ascontiguousarray(out, dtype=np.float32)


# revision 26
# speedup vs baseline: 1.1790x; 1.0078x over previous
"""Bahdanau-attention score kernel (softmax(v . tanh(W[h;enc]+b))) for 8 TRN2 cores.

Self-contained: hardcodes shapes B=32, S=2048, ENC2=600, DD=900.
Sharding: data-parallel over batch (4 batches/core), weights replicated.

Design (v6):
- Host prep: enc is cast to fp16 and TRANSPOSED on the host into
  encT [604, 8192] (rows 600..603 carry a one-hot(batch), so the
  h-projection -- computed on host and appended to We^T rows -- lands
  via 4 extra contraction rows). Host transposition lets every device
  load be a plain row-major DMA: measured ~178 GB/s effective vs
  ~76 GB/s for the on-chip xbar-transpose path (which also corrupts
  its 512-row tail and serializes per queue).
- PE runs the 10 contraction matmuls per 128-row tile (fp16 operands,
  f32 PSUM, N split 512+388 for the PSUM bank limit).
- ACT drains PSUM with tanh (fp16 out); DVE does one fused
  scalar_tensor_tensor: junk=(z*1)*v_rep with accum_out=scores column.
- Softmax is per batch and overlapped: transpose+exp after the batch's
  last tile, normalize via PE ones-matmul sum + reciprocal + PE
  broadcast (no DRAM roundtrip), per-batch output DMA on the idle
  GPSIMD queue. Only batch 3's tail is exposed.
"""

import os

import numpy as np

import concourse.bass as bass  # noqa: F401
import concourse.mybir as mybir
import concourse.tile as tile
from concourse import bacc
from concourse.bass_utils import run_bass_kernel_spmd
from concourse.masks import make_identity

F32 = mybir.dt.float32
F16 = mybir.dt.float16
AF = mybir.ActivationFunctionType
ALU = mybir.AluOpType
AX = mybir.AxisListType

NCORES = 8
B, S, E2, DD = 32, 2048, 600, 900
BL = B // NCORES            # 4 batches per core
SROWS = BL * S              # 8192 s-rows per core
P = 128
NTIL = SROWS // P           # 64 score tiles/columns
TPB = S // P                # 16 tiles per batch
NCH = 5                     # e chunks of 128 (last: 88 enc + 4 one-hot)
K4 = 92                     # chunk-4 contraction rows
KE = 512 + K4               # 604 encT rows

K_TILES = int(os.environ.get("K_TILES", NTIL))


def build():
    nc = bacc.Bacc("TRN2", target_bir_lowering=False)
    encT_ext = nc.dram_tensor("encT", [KE, SROWS], F16, kind="ExternalInput")
    wcat_ext = nc.dram_tensor("wcat", [KE, DD], F16, kind="ExternalInput")
    v_ext = nc.dram_tensor("v", [1, DD], F16, kind="ExternalInput")
    out_ext = nc.dram_tensor("out", [BL, S], F32, kind="ExternalOutput")

    with tile.TileContext(nc) as tc:
        with (
            tc.tile_pool(name="stat", bufs=1) as stat,
            tc.tile_pool(name="encp", bufs=BL) as encp,
            tc.tile_pool(name="zp", bufs=3) as zp,
            tc.tile_pool(name="jp", bufs=2) as jp,
            tc.tile_pool(name="ps_e", bufs=3, space="PSUM") as ps_e,
            tc.tile_pool(name="ps_t", bufs=2, space="PSUM") as ps_t,
        ):
            # --------- weights split across the three DMA queue heads -----
            rhs_main = stat.tile([P, 4, DD], F16)
            rhs4 = stat.tile([K4, DD], F16)
            v_sb = stat.tile([1, DD], F16)
            v_rep = stat.tile([P, DD], F16)
            ones_r = stat.tile([1, P], F16)
            nc.vector.memset(ones_r[:, :], 1.0)

            def w_dma(eng, h):
                # halves so the single-transfer latency (~22 GB/s/engine)
                # doesn't gate the first tiles
                for (no, nn) in ((0, 450), (450, 450)):
                    eng.dma_start(out=rhs_main[:, h, no:no + nn],
                                  in_=wcat_ext.ap()[h * P:(h + 1) * P,
                                                    no:no + nn])

            nc.gpsimd.dma_start(out=v_sb[:, :], in_=v_ext.ap())

            # PE warm-up: keep the HAM activity monitor busy during the
            # startup DMA wait so the first real matmuls run at 2.4 GHz
            warm = ps_t.tile([P, P], F32, tag="tp", name="warm")
            for _ in range(64):
                nc.tensor.matmul(warm[0:P, 0:P], ones_r[:, :], ones_r[:, :])

            # v broadcast across partitions: psum[p, :] = ones[p] * v
            vps = ps_e.tile([P, DD], F32, tag="ep")
            for (no, nn) in ((0, 512), (512, 388)):
                nc.tensor.matmul(vps[:, no:no + nn], ones_r[:, :],
                                 v_sb[:, no:no + nn])
            nc.scalar.copy(v_rep[:, :], vps[:, :])

            # --------- enc tiles + weights: row-major loads on SP ---------
            # Early batches are split into column pieces: a whole [128,2048]
            # transfer has ~20us single-engine latency, which would gate the
            # first tiles even though aggregate bandwidth is fine. Weight
            # halves are interleaved with batch 0's head pieces in the order
            # the first tile consumes them.
            enc_tiles = {}
            for b in range(BL):
                for c in range(NCH):
                    r0, rn = (c * P, P) if c < 4 else (512, K4)
                    enc_tiles[(b, c)] = encp.tile(
                        [P, S], F16, tag=f"enc{c}", name=f"enc{b}_{c}"
                    )

            def enc_dma(b, c, s0, s1):
                r0, rn = (c * P, P) if c < 4 else (512, K4)
                nc.sync.dma_start(
                    out=enc_tiles[(b, c)][0:rn, s0:s1],
                    in_=encT_ext.ap()[r0:r0 + rn, b * S + s0:b * S + s1],
                )

            w_dma(nc.sync, 0)
            enc_dma(0, 0, 0, 256)
            w_dma(nc.sync, 1)
            enc_dma(0, 1, 0, 256)
            w_dma(nc.sync, 2)
            enc_dma(0, 2, 0, 256)
            w_dma(nc.sync, 3)
            enc_dma(0, 3, 0, 256)
            for (no, nn) in ((0, 450), (450, 450)):
                nc.sync.dma_start(out=rhs4[:, no:no + nn],
                                  in_=wcat_ext.ap()[512:KE, no:no + nn])
            enc_dma(0, 4, 0, 256)
            col_splits = {
                0: (1024, S),
                1: (1024, S),
            }
            for b in range(BL):
                for c in range(NCH):
                    s0 = 256 if b == 0 else 0
                    for s1 in col_splits.get(b, (S,)):
                        if s1 > s0:
                            enc_dma(b, c, s0, s1)
                            s0 = s1

            # ---------------- constants ----------------
            ident_f = stat.tile([P, P], F32)
            make_identity(nc, ident_f[:, :])
            ones16 = stat.tile([TPB, 1], F32)
            nc.vector.memset(ones16[:, :], 1.0)
            ones1x16 = stat.tile([1, TPB], F32)
            nc.vector.memset(ones1x16[:, :], 1.0)

            scores = stat.tile([P, NTIL], F32)
            e1 = stat.tile([TPB, BL, P], F32)
            rs = stat.tile([TPB, BL], F32)
            rbi = stat.tile([1, BL], F32)
            outf = stat.tile([TPB, BL, P], F32)

            # ---------------- per-batch softmax pieces ----------------
            def emit_exp(b):
                c0 = b * TPB
                pst = ps_t.tile([P, P], F32, tag="tp", name=f"pst{b}")
                nc.tensor.transpose(
                    pst[0:TPB, :], scores[:, c0:c0 + TPB], ident_f[:, :]
                )
                nc.scalar.activation(
                    e1[:, b, :], pst[0:TPB, :], AF.Exp,
                    accum_out=rs[:, b:b + 1],
                )

            def emit_tail(b):
                zb = ps_t.tile([P, P], F32, tag="tp", name=f"zb{b}")
                nc.tensor.matmul(zb[0:1, 0:1], ones16[:, :], rs[:, b:b + 1])
                nc.vector.reciprocal(rbi[:, b:b + 1], zb[0:1, 0:1])
                rfacp = ps_t.tile([P, P], F32, tag="tp", name=f"rf{b}")
                nc.tensor.matmul(rfacp[0:TPB, 0:1], ones1x16[:, :],
                                 rbi[:, b:b + 1])
                nc.vector.tensor_scalar_mul(
                    outf[:, b, :], e1[:, b, :], rfacp[0:TPB, 0:1]
                )
                # last batch rides SP (idle by then, no SWDGE drain cost)
                eng = nc.sync if b == BL - 1 else nc.gpsimd
                eng.dma_start(
                    out=out_ext.ap()[b:b + 1, :].rearrange(
                        "b (t p) -> (b t) p", p=P),
                    in_=outf[:, b, :],
                )

            # ---------------- main loop ----------------
            for t in range(K_TILES):
                b, ti = divmod(t, TPB)
                eps = ps_e.tile([P, DD], F32, tag="ep")
                for c in range(NCH):
                    et = enc_tiles[(b, c)]
                    kk = P if c < 4 else K4
                    rr = rhs_main[:, c, :] if c < 4 else rhs4[:, :]
                    for (no, nn) in ((0, 512), (512, 388)):
                        nc.tensor.matmul(
                            eps[:, no:no + nn],
                            et[0:kk, ti * P:(ti + 1) * P],
                            rr[:, no:no + nn],
                            start=(c == 0), stop=(c == NCH - 1),
                        )
                z = zp.tile([P, DD], F16, tag="z")
                nc.scalar.activation(z[:, :], eps[:, :], AF.Tanh)
                junk = jp.tile([P, DD], F16, tag="junk")
                nc.vector.scalar_tensor_tensor(
                    out=junk[:, :], in0=z[:, :], scalar=1.0, in1=v_rep[:, :],
                    op0=ALU.mult, op1=ALU.mult,
                    accum_out=scores[:, t:t + 1],
                )

                if K_TILES != NTIL:
                    continue
                # overlapped softmax for the previous batch
                if b >= 1 and ti == 1:
                    emit_exp(b - 1)
                if b >= 1 and ti == 6:
                    emit_tail(b - 1)

            if K_TILES < NTIL:
                return nc

            emit_exp(BL - 1)
            emit_tail(BL - 1)
    return nc


_CACHE = {}


def _get_nc():
    if "nc" not in _CACHE:
        nc = build()
        nc.compile()
        _CACHE["nc"] = nc
    return _CACHE["nc"]


def make_in_maps(hidden, encoder_outputs, attn_W, attn_b, v):
    hidden = np.asarray(hidden, dtype=np.float32)
    encoder_outputs = np.asarray(encoder_outputs, dtype=np.float32)
    attn_W = np.asarray(attn_W, dtype=np.float32)
    attn_b = np.asarray(attn_b, dtype=np.float32)
    v = np.asarray(v, dtype=np.float32)

    WeT = np.ascontiguousarray(attn_W[:, DD:].T)          # [600, 900]
    hb_all = hidden @ attn_W[:, :DD].T + attn_b           # [32, 900]
    v16 = v.astype(np.float16).reshape(1, DD)

    in_maps = []
    for c in range(NCORES):
        bs = slice(c * BL, (c + 1) * BL)
        encT = np.zeros((KE, SROWS), dtype=np.float16)
        encT[:E2, :] = encoder_outputs[bs].reshape(SROWS, E2).T
        for b in range(BL):
            encT[E2 + b, b * S:(b + 1) * S] = 1.0
        wcat = np.concatenate([WeT, hb_all[bs]], axis=0).astype(np.float16)
        in_maps.append({
            "encT": encT,
            "wcat": np.ascontiguousarray(wcat),
            "v": v16,
        })
    return in_maps


def run(in_maps, trace=False, **kw):
    nc = _get_nc()
    return run_bass_kernel_spmd(nc, in_maps, core_ids=list(range(NCORES)),
                                trace=trace, **kw)


def kernel(hidden, encoder_outputs, attn_W, attn_b, v):
    in_maps = make_in_maps(hidden, encoder_outputs, attn_W, attn_b, v)
    try:
        res = run(in_maps)
    except Exception:
        # transient device states (e.g. a previously wedged core) sometimes
        # clear on retry
        res = run(in_maps)
    out = np.concatenate([res.results[c]["out"] for c in range(NCORES)], axis=0)
    return np.ascontiguousarray(out, dtype=np.float32)


# revision 27
# speedup vs baseline: 1.1794x; 1.0004x over previous
"""Bahdanau-attention score kernel (softmax(v . tanh(W[h;enc]+b))) for 8 TRN2 cores.

Self-contained: hardcodes shapes B=32, S=2048, ENC2=600, DD=900.
Sharding: data-parallel over batch (4 batches/core), weights replicated.

Design (v6):
- Host prep: enc is cast to fp16 and TRANSPOSED on the host into
  encT [604, 8192] (rows 600..603 carry a one-hot(batch), so the
  h-projection -- computed on host and appended to We^T rows -- lands
  via 4 extra contraction rows). Host transposition lets every device
  load be a plain row-major DMA: measured ~178 GB/s effective vs
  ~76 GB/s for the on-chip xbar-transpose path (which also corrupts
  its 512-row tail and serializes per queue).
- PE runs the 10 contraction matmuls per 128-row tile (fp16 operands,
  f32 PSUM, N split 512+388 for the PSUM bank limit).
- ACT drains PSUM with tanh (fp16 out); DVE does one fused
  scalar_tensor_tensor: junk=(z*1)*v_rep with accum_out=scores column.
- Softmax is per batch and overlapped: transpose+exp after the batch's
  last tile, normalize via PE ones-matmul sum + reciprocal + PE
  broadcast (no DRAM roundtrip), per-batch output DMA on the idle
  GPSIMD queue. Only batch 3's tail is exposed.
"""

import os

import numpy as np

import concourse.bass as bass  # noqa: F401
import concourse.mybir as mybir
import concourse.tile as tile
from concourse import bacc
from concourse.bass_utils import run_bass_kernel_spmd
from concourse.masks import make_identity

F32 = mybir.dt.float32
F16 = mybir.dt.float16
AF = mybir.ActivationFunctionType
ALU = mybir.AluOpType
AX = mybir.AxisListType

NCORES = 8
B, S, E2, DD = 32, 2048, 600, 900
BL = B // NCORES            # 4 batches per core
SROWS = BL * S              # 8192 s-rows per core
P = 128
NTIL = SROWS // P           # 64 score tiles/columns
TPB = S // P                # 16 tiles per batch
NCH = 5                     # e chunks of 128 (last: 88 enc + 4 one-hot)
K4 = 92                     # chunk-4 contraction rows
KE = 512 + K4               # 604 encT rows

K_TILES = int(os.environ.get("K_TILES", NTIL))


def build():
    nc = bacc.Bacc("TRN2", target_bir_lowering=False)
    encT_ext = nc.dram_tensor("encT", [KE, SROWS], F16, kind="ExternalInput")
    wcat_ext = nc.dram_tensor("wcat", [KE, DD], F16, kind="ExternalInput")
    v_ext = nc.dram_tensor("v", [1, DD], F16, kind="ExternalInput")
    out_ext = nc.dram_tensor("out", [BL, S], F32, kind="ExternalOutput")

    with tile.TileContext(nc) as tc:
        with (
            tc.tile_pool(name="stat", bufs=1) as stat,
            tc.tile_pool(name="encp", bufs=BL) as encp,
            tc.tile_pool(name="zp", bufs=3) as zp,
            tc.tile_pool(name="jp", bufs=2) as jp,
            tc.tile_pool(name="ps_e", bufs=3, space="PSUM") as ps_e,
            tc.tile_pool(name="ps_t", bufs=2, space="PSUM") as ps_t,
        ):
            # --------- weights split across the three DMA queue heads -----
            rhs_main = stat.tile([P, 4, DD], F16)
            rhs4 = stat.tile([K4, DD], F16)
            v_sb = stat.tile([1, DD], F16)
            v_rep = stat.tile([P, DD], F16)
            ones_r = stat.tile([1, P], F16)
            nc.vector.memset(ones_r[:, :], 1.0)

            def w_dma(eng, h):
                # halves so the single-transfer latency (~22 GB/s/engine)
                # doesn't gate the first tiles
                for (no, nn) in ((0, 450), (450, 450)):
                    eng.dma_start(out=rhs_main[:, h, no:no + nn],
                                  in_=wcat_ext.ap()[h * P:(h + 1) * P,
                                                    no:no + nn])

            nc.gpsimd.dma_start(out=v_sb[:, :], in_=v_ext.ap())

            # PE warm-up: keep the HAM activity monitor busy during the
            # startup DMA wait so the first real matmuls run at 2.4 GHz
            warm = ps_t.tile([P, P], F32, tag="tp", name="warm")
            for _ in range(44):
                nc.tensor.matmul(warm[0:P, 0:P], ones_r[:, :], ones_r[:, :])

            # v broadcast across partitions: psum[p, :] = ones[p] * v
            vps = ps_e.tile([P, DD], F32, tag="ep")
            for (no, nn) in ((0, 512), (512, 388)):
                nc.tensor.matmul(vps[:, no:no + nn], ones_r[:, :],
                                 v_sb[:, no:no + nn])
            nc.scalar.copy(v_rep[:, :], vps[:, :])

            # --------- enc tiles + weights: row-major loads on SP ---------
            # Early batches are split into column pieces: a whole [128,2048]
            # transfer has ~20us single-engine latency, which would gate the
            # first tiles even though aggregate bandwidth is fine. Weight
            # halves are interleaved with batch 0's head pieces in the order
            # the first tile consumes them.
            enc_tiles = {}
            for b in range(BL):
                for c in range(NCH):
                    r0, rn = (c * P, P) if c < 4 else (512, K4)
                    enc_tiles[(b, c)] = encp.tile(
                        [P, S], F16, tag=f"enc{c}", name=f"enc{b}_{c}"
                    )

            def enc_dma(b, c, s0, s1):
                r0, rn = (c * P, P) if c < 4 else (512, K4)
                nc.sync.dma_start(
                    out=enc_tiles[(b, c)][0:rn, s0:s1],
                    in_=encT_ext.ap()[r0:r0 + rn, b * S + s0:b * S + s1],
                )

            w_dma(nc.sync, 0)
            enc_dma(0, 0, 0, 256)
            w_dma(nc.sync, 1)
            enc_dma(0, 1, 0, 256)
            w_dma(nc.sync, 2)
            enc_dma(0, 2, 0, 256)
            w_dma(nc.sync, 3)
            enc_dma(0, 3, 0, 256)
            for (no, nn) in ((0, 450), (450, 450)):
                nc.sync.dma_start(out=rhs4[:, no:no + nn],
                                  in_=wcat_ext.ap()[512:KE, no:no + nn])
            enc_dma(0, 4, 0, 256)
            col_splits = {
                0: (1024, S),
                1: (1024, S),
            }
            for b in range(BL):
                for c in range(NCH):
                    s0 = 256 if b == 0 else 0
                    for s1 in col_splits.get(b, (S,)):
                        if s1 > s0:
                            enc_dma(b, c, s0, s1)
                            s0 = s1

            # ---------------- constants ----------------
            ident_f = stat.tile([P, P], F32)
            make_identity(nc, ident_f[:, :])
            ones16 = stat.tile([TPB, 1], F32)
            nc.vector.memset(ones16[:, :], 1.0)
            ones1x16 = stat.tile([1, TPB], F32)
            nc.vector.memset(ones1x16[:, :], 1.0)

            scores = stat.tile([P, NTIL], F32)
            e1 = stat.tile([TPB, BL, P], F32)
            rs = stat.tile([TPB, BL], F32)
            rbi = stat.tile([1, BL], F32)
            outf = stat.tile([TPB, BL, P], F32)

            # ---------------- per-batch softmax pieces ----------------
            def emit_exp(b):
                c0 = b * TPB
                pst = ps_t.tile([P, P], F32, tag="tp", name=f"pst{b}")
                nc.tensor.transpose(
                    pst[0:TPB, :], scores[:, c0:c0 + TPB], ident_f[:, :]
                )
                nc.scalar.activation(
                    e1[:, b, :], pst[0:TPB, :], AF.Exp,
                    accum_out=rs[:, b:b + 1],
                )

            def emit_tail(b):
                zb = ps_t.tile([P, P], F32, tag="tp", name=f"zb{b}")
                nc.tensor.matmul(zb[0:1, 0:1], ones16[:, :], rs[:, b:b + 1])
                nc.vector.reciprocal(rbi[:, b:b + 1], zb[0:1, 0:1])
                rfacp = ps_t.tile([P, P], F32, tag="tp", name=f"rf{b}")
                nc.tensor.matmul(rfacp[0:TPB, 0:1], ones1x16[:, :],
                                 rbi[:, b:b + 1])
                nc.vector.tensor_scalar_mul(
                    outf[:, b, :], e1[:, b, :], rfacp[0:TPB, 0:1]
                )
                # last batch rides SP (idle by then, no SWDGE drain cost)
                eng = nc.sync if b == BL - 1 else nc.gpsimd
                eng.dma_start(
                    out=out_ext.ap()[b:b + 1, :].rearrange(
                        "b (t p) -> (b t) p", p=P),
                    in_=outf[:, b, :],
                )

            # ---------------- main loop ----------------
            for t in range(K_TILES):
                b, ti = divmod(t, TPB)
                eps = ps_e.tile([P, DD], F32, tag="ep")
                for c in range(NCH):
                    et = enc_tiles[(b, c)]
                    kk = P if c < 4 else K4
                    rr = rhs_main[:, c, :] if c < 4 else rhs4[:, :]
                    for (no, nn) in ((0, 512), (512, 388)):
                        nc.tensor.matmul(
                            eps[:, no:no + nn],
                            et[0:kk, ti * P:(ti + 1) * P],
                            rr[:, no:no + nn],
                            start=(c == 0), stop=(c == NCH - 1),
                        )
                z = zp.tile([P, DD], F16, tag="z")
                nc.scalar.activation(z[:, :], eps[:, :], AF.Tanh)
                junk = jp.tile([P, DD], F16, tag="junk")
                nc.vector.scalar_tensor_tensor(
                    out=junk[:, :], in0=z[:, :], scalar=1.0, in1=v_rep[:, :],
                    op0=ALU.mult, op1=ALU.mult,
                    accum_out=scores[:, t:t + 1],
                )

                if K_TILES != NTIL:
                    continue
                # overlapped softmax for the previous batch
                if b >= 1 and ti == 1:
                    emit_exp(b - 1)
                if b >= 1 and ti == 6:
                    emit_tail(b - 1)

            if K_TILES < NTIL:
                return nc

            emit_exp(BL - 1)
            emit_tail(BL - 1)
    return nc


_CACHE = {}


def _get_nc():
    if "nc" not in _CACHE:
        nc = build()
        nc.compile()
        _CACHE["nc"] = nc
    return _CACHE["nc"]


def make_in_maps(hidden, encoder_outputs, attn_W, attn_b, v):
    hidden = np.asarray(hidden, dtype=np.float32)
    encoder_outputs = np.asarray(encoder_outputs, dtype=np.float32)
    attn_W = np.asarray(attn_W, dtype=np.float32)
    attn_b = np.asarray(attn_b, dtype=np.float32)
    v = np.asarray(v, dtype=np.float32)

    WeT = np.ascontiguousarray(attn_W[:, DD:].T)          # [600, 900]
    hb_all = hidden @ attn_W[:, :DD].T + attn_b           # [32, 900]
    v16 = v.astype(np.float16).reshape(1, DD)

    in_maps = []
    for c in range(NCORES):
        bs = slice(c * BL, (c + 1) * BL)
        encT = np.zeros((KE, SROWS), dtype=np.float16)
        encT[:E2, :] = encoder_outputs[bs].reshape(SROWS, E2).T
        for b in range(BL):
            encT[E2 + b, b * S:(b + 1) * S] = 1.0
        wcat = np.concatenate([WeT, hb_all[bs]], axis=0).astype(np.float16)
        in_maps.append({
            "encT": encT,
            "wcat": np.ascontiguousarray(wcat),
            "v": v16,
        })
    return in_maps


def run(in_maps, trace=False, **kw):
    nc = _get_nc()
    return run_bass_kernel_spmd(nc, in_maps, core_ids=list(range(NCORES)),
                                trace=trace, **kw)


def kernel(hidden, encoder_outputs, attn_W, attn_b, v):
    in_maps = make_in_maps(hidden, encoder_outputs, attn_W, attn_b, v)
    try:
        res = run(in_maps)
    except Exception:
        # transient device states (e.g. a previously wedged core) sometimes
        # clear on retry
        res = run(in_maps)
    out = np.concatenate([res.results[c]["out"] for c in range(NCORES)], axis=0)
    return np.ascontiguousarray(out, dtype=np.float32)
